# revision 1
# baseline (speedup 1.0000x reference)
"""Binary-cross-entropy custom loss on 8 Trainium2 NeuronCores.

reference math:
    ll   = lab*log_sigmoid(p) + (1-lab)*log_sigmoid(-p) = lab*p - softplus(p)
    loss = -sum(ll) / ((1 + neg) * pos),  pos = sum(lab), neg = N - pos

Data-parallel over N=2^24, 2M elements per core.  Per-core engine split:
  ACT : e = exp(p); softplus = ln(e + 1) with accum_out -> per-partition sums
        (this build has no softplus ACT table; exp/ln share one table set,
        manually preloaded so the insertion pass emits no per-tile reloads)
  DVE : prod = lab * p (bf16 out, one pass) + per-tile pos counts
  PE  : ones-vector matmuls accumulate sum(lab*p) into PSUM
  host: float64 scalar combine of the 8 cores' partials

Inputs are packed host-side into one [P, 16384] f32 tensor per core: for
each tile, Fi/2 f32 lanes of p as fp16 followed by Fi/2 lanes of labels
as fp16 (lossless 0/1).  One dma_start per tile (single semaphore -- the
CoreV3 ISA has one sync-wait slot per instruction).  fp16 p quantization
adds ~1e-6 relative error to the loss (sums of ~16M near-random-sign
rounding errors) while halving DMA traffic and enabling the DVE 2x 16-bit
mode.  Tile sizes ramp up/down (small first tiles so compute starts
sooner, small last tile so the tail is not gated by a 3 MB transfer).
"""
import sys

if "/opt/trn_rl_repo" not in sys.path:
    sys.path.insert(0, "/opt/trn_rl_repo")

import ml_dtypes
import numpy as np

import concourse.bacc as bacc
import concourse.bass as bass
import concourse.mybir as mybir
import concourse.tile as tile
from concourse.bass_utils import run_bass_kernel_spmd
from concourse.hw_specs import get_activation_tables

N = 16777216
N_CORES = 8
P = 128
TILES = [1024, 2048, 2048, 3584, 3584, 3584, 512]  # per-tile free-dim Fi
assert sum(TILES) * P * N_CORES == N
MM = 512  # matmul free-dim chunk (one PSUM bank)
TOTALC = sum(TILES)  # f32 lanes per partition row (bf16 p + bf16 lab)

_NC_CACHE = None


def _light_drain_and_barrier(self, tick_clock, wait_clock):
    """TileContext exit with the semaphore-clear cascade and second barrier
    dropped (~2us): the Bass preamble re-clears semaphores on each launch,
    so the exit-side clear is redundant for this kernel (verified over
    repeated executions)."""
    from concourse.tile import ScopedClock

    drain_inst = self.nc.sync.drain()
    wait_clock.add_sem_waits(drain_inst.ins, ScopedClock({None: tick_clock.global_clock}))
    self.nc.all_engine_barrier()
    assert self.sems is not None
    popped = self.nc._tile_sem_poison_stack.pop()
    assert popped is self._sem_poison


def build_nc(tiles=None):
    """Build the (single-program, 8-core SPMD) Bass module."""
    tiles = TILES if tiles is None else tiles
    totalc = sum(tiles)
    T = len(tiles)
    nc = bacc.Bacc(
        "TRN2",
        target_bir_lowering=False,
        debug=False,
        enable_asserts=False,
        num_devices=N_CORES,
    )
    data_dram = nc.dram_tensor("data", [P, totalc], mybir.dt.float32, kind="ExternalInput").ap()
    out_dram = nc.dram_tensor("partials", [P, 3], mybir.dt.float32, kind="ExternalOutput").ap()

    orig_drain = tile.TileContext._drain_and_barrier
    tile.TileContext._drain_and_barrier = _light_drain_and_barrier
    try:
        _build_body(nc, tiles, data_dram, out_dram)
    finally:
        tile.TileContext._drain_and_barrier = orig_drain
    nc.compile()  # bacc legalization: split multi-waits via event semaphores
    return nc


def _build_body(nc, tiles, data_dram, out_dram):
    T = len(tiles)
    with tile.TileContext(nc) as tc:
        # Preload the one ACT table set containing BOTH exp and ln; the
        # auto-insertion pass then sees every activation's table resident.
        act_tables = list(get_activation_tables(nc.m.arch).keys())
        nle_id = act_tables.index("natural_log_exp_and_others")
        nc.scalar.add_instruction(mybir.InstLoadActFuncSet(
            name=nc.get_next_instruction_name(), ins=[], outs=[],
            act_func_set_id=nle_id,
        ))
        with tc.tile_pool(name="io", bufs=5) as io_pool, \
             tc.tile_pool(name="ajunk", bufs=3) as act_junk, \
             tc.tile_pool(name="vjunk", bufs=3) as dve_junk, \
             tc.tile_pool(name="psum", bufs=1, space="PSUM") as psum_pool, \
             tc.tile_pool(name="acc", bufs=1) as acc_pool:
            sp_cols = acc_pool.tile([P, T], mybir.dt.float32)
            pos_cols = acc_pool.tile([P, T], mybir.dt.float32)
            sums = acc_pool.tile([P, 3], mybir.dt.float32)
            ones_bf = acc_pool.tile([P, 1], mybir.dt.float16)
            ts_dummy = acc_pool.tile([P, 1], mybir.dt.float16)
            nc.vector.memset(ones_bf[:], 1.0)
            nc.vector.memset(sums[:], 0.0)
            psum_lp = psum_pool.tile([1, MM], mybir.dt.float32)
            fmax = max(tiles)
            n_mms = sum(f // MM for f in tiles)
            c0 = 0
            mm_idx = 0
            for i, F in enumerate(tiles):
                w = F
                data_t = io_pool.tile([P, fmax], mybir.dt.float32,
                                      name="data_t")
                nc.sync.dma_start(data_t[:, 0:w], data_dram[:, c0:c0 + w])
                p_t = data_t[:, 0:F // 2].bitcast(mybir.dt.float16)  # [P, F]
                lab_bf = data_t[:, F // 2:w].bitcast(mybir.dt.float16)  # [P, F]

                e_t = act_junk.tile([P, fmax], mybir.dt.float16, name="e_t")
                nc.scalar.activation(e_t[:, 0:F], p_t, mybir.ActivationFunctionType.Exp)
                sp_junk = act_junk.tile([P, fmax], mybir.dt.float32, name="sp_junk")
                nc.scalar.activation(
                    sp_junk[:, 0:F],
                    e_t[:, 0:F],
                    mybir.ActivationFunctionType.Ln,
                    bias=1.0,
                    accum_out=sp_cols[:, i:i + 1],
                )
                prod_bf = dve_junk.tile([P, fmax], mybir.dt.float16, name="prod_bf")
                nc.vector.tensor_mul(prod_bf[:, 0:F], lab_bf, p_t)
                nc.vector.tensor_scalar(
                    out=ts_dummy.broadcast_to((P, F)),
                    in0=lab_bf,
                    scalar1=1.0,
                    scalar2=None,
                    op0=mybir.AluOpType.mult,
                    op1=mybir.AluOpType.add,
                    accum_out=pos_cols[:, i:i + 1],
                )
                for j in range(F // MM):
                    nc.tensor.matmul(
                        psum_lp[:],
                        ones_bf[:],
                        prod_bf[:, j * MM:(j + 1) * MM],
                        start=mm_idx == 0,
                        stop=mm_idx == n_mms - 1,
                        skip_group_check=True,
                    )
                    mm_idx += 1
                c0 += w
            # Tail: per-partition softplus sums -> col 0; scalar lab*p sum
            # (partition 0 only) -> col 1; per-partition lab counts -> col 2.
            nc.vector.reduce_sum(out=sums[:, 0:1], in_=sp_cols[:], axis=mybir.AxisListType.X)
            nc.vector.reduce_sum(out=sums[0:1, 1:2], in_=psum_lp[:], axis=mybir.AxisListType.X)
            nc.vector.reduce_sum(out=sums[:, 2:3], in_=pos_cols[:], axis=mybir.AxisListType.X)
            nc.sync.dma_start(out_dram[:], sums[:])


def get_nc():
    global _NC_CACHE
    if _NC_CACHE is None:
        _NC_CACHE = build_nc()
    return _NC_CACHE


def pack_inputs(pv, lb, tiles):
    """pv, lb: [cores, elems] -> packed bf16-pair [cores, P, totalc] f32."""
    n_cores = pv.shape[0]
    totalc = sum(tiles)
    data = np.empty((n_cores, P, totalc), dtype=np.float32)
    e0 = 0
    c0 = 0
    for F in tiles:
        ne = P * F
        data[:, :, c0:c0 + F // 2] = (
            pv[:, e0:e0 + ne].reshape(n_cores, P, F)
            .astype(np.float16).view(np.float32)
        )
        data[:, :, c0 + F // 2:c0 + F] = (
            lb[:, e0:e0 + ne].reshape(n_cores, P, F)
            .astype(np.float16).view(np.float32)
        )
        e0 += ne
        c0 += F
    return data


def shard_inputs(predicted_values, labels):
    pv = np.ascontiguousarray(predicted_values, dtype=np.float32).reshape(N_CORES, -1)
    lb = np.ascontiguousarray(labels, dtype=np.int32).reshape(N_CORES, -1)
    data = pack_inputs(pv, lb, TILES)
    return [{"data": data[c]} for c in range(N_CORES)]


def combine(results):
    """results: list of 8 dicts with 'partials' [128,3] -> loss [1] f32.

    col 0: per-partition softplus sums; col 1 row 0: sum(lab*p);
    col 2: per-partition lab counts."""
    s_sp = s_lp = pos = 0.0
    for r in results:
        part = r["partials"].astype(np.float64)
        s_sp += part[:, 0].sum()
        s_lp += part[0, 1]
        pos += part[:, 2].sum()
    neg = float(N) - pos
    loss = (s_sp - s_lp) / ((1.0 + neg) * pos)
    return np.array([loss], dtype=np.float32)


_RUNNER = None


def _get_runner():
    """Build the SPMD executable ONCE and reuse it: run_bass_kernel_spmd
    constructs a fresh jax.jit per call, which recompiles (~1 min) on every
    invocation.  This is the same dispatch run_bass_via_pjrt performs for
    the multi-core axon path, with the jitted callable cached."""
    global _RUNNER
    if _RUNNER is not None:
        return _RUNNER
    import jax
    from jax.sharding import Mesh, PartitionSpec
    from jax.experimental.shard_map import shard_map

    from concourse import bass2jax, mybir as mb

    nc = get_nc()
    bass2jax.install_neuronx_cc_hook()
    assert nc.dbg_addr is None
    partition_name = nc.partition_id_tensor.name if nc.partition_id_tensor else None

    in_names, out_names, out_avals, zero_outs = [], [], [], []
    for alloc in nc.m.functions[0].allocations:
        if not isinstance(alloc, mb.MemoryLocationSet):
            continue
        name = alloc.memorylocations[0].name
        if alloc.kind == "ExternalInput":
            if name != partition_name:
                in_names.append(name)
        elif alloc.kind == "ExternalOutput":
            shape = tuple(alloc.tensor_shape)
            dtype = mb.dt.np(alloc.dtype)
            out_names.append(name)
            out_avals.append(jax.core.ShapedArray(shape, dtype))
            zero_outs.append(np.zeros(shape, dtype))
    n_params = len(in_names)
    donate = tuple(range(n_params, n_params + len(out_avals)))
    all_in_names = list(in_names) + list(out_names)
    if partition_name is not None:
        all_in_names.append(partition_name)

    def _body(*args):
        operands = list(args)
        if partition_name is not None:
            operands.append(bass2jax.partition_id_tensor())
        outs = bass2jax._bass_exec_p.bind(
            *operands,
            out_avals=tuple(out_avals),
            in_names=tuple(all_in_names),
            out_names=tuple(out_names),
            lowering_input_output_aliases=(),
            sim_require_finite=True,
            sim_require_nnan=True,
            nc=nc,
        )
        return tuple(outs)

    devices = jax.devices()[:N_CORES]
    mesh = Mesh(np.asarray(devices), ("core",))
    nio = n_params + len(out_avals)
    sharded = jax.jit(
        shard_map(
            _body,
            mesh=mesh,
            in_specs=(PartitionSpec("core"),) * nio,
            out_specs=(PartitionSpec("core"),) * len(out_names),
            check_rep=False,
        ),
        donate_argnums=donate,
        keep_unused=True,
    )

    def run(in_maps):
        concat_in = [
            np.concatenate([np.asarray(m[name]) for m in in_maps], axis=0)
            for name in in_names
        ]
        concat_zeros = [
            np.zeros((N_CORES * z.shape[0], *z.shape[1:]), z.dtype)
            for z in zero_outs
        ]
        out_arrs = sharded(*concat_in, *concat_zeros)
        return [
            {
                name: np.asarray(out_arrs[k]).reshape(N_CORES, *out_avals[k].shape)[c]
                for k, name in enumerate(out_names)
            }
            for c in range(N_CORES)
        ]

    _RUNNER = run
    return _RUNNER


def kernel(predicted_values, labels):
    assert predicted_values.shape == (N,) and labels.shape == (N,)
    in_maps = shard_inputs(predicted_values, labels)
    results = _get_runner()(in_maps)
    return combine(results)


if __name__ == "__main__":
    rng = np.random.default_rng(0)
    pv = rng.standard_normal(N).astype(np.float32)
    lb = rng.integers(0, 2, size=N).astype(np.int32)
    out = kernel(pv, lb)
    print("loss:", out)



# revision 8
# speedup vs baseline: 1.2436x; 1.2436x over previous
"""Binary-cross-entropy custom loss on 8 Trainium2 NeuronCores.

reference math:
    ll   = lab*log_sigmoid(p) + (1-lab)*log_sigmoid(-p)
    -ll  = softplus((1-2*lab)*p) = softplus(s)      (sign fold)
    loss = sum(softplus(s)) / ((1 + neg) * pos),  pos = sum(lab), neg = N - pos

Data-parallel over N=2^24, 2M elements per core.  Host packs s = fp16(p)
with the label XOR'd into the sign bit (lossless fold of the label into
the value whose softplus we need), plus the raw labels bit-packed
(8 labels/byte).  Per-core DMA: 4.25 MB (vs 16 MB raw f32/int32).

Per-core engine split:
  ACT : e = exp(s) per tile (the only full-rate pass, ~0.83 ns/col), then
        ONE ln over the 1024 group-products with accum_out
  DVE : product tree  z = prod_16(1+e)/4^16  via tensor_scalar +
        scalar_tensor_tensor + 3 tensor_tensor levels (all 16-bit 2x/4x
        modes), plus SWAR popcount of the label bits for pos
  PE  : single fp32 matmul with a ones vector = partition reduce of the
        [128,3] partials -> PSUM [1,3]
  out : one 12-byte DMA -> host float64 scalar combine

sum(softplus) = sum(ln z) + N*ln(4): each (1+e) carries a 1/4 scale so
bf16 group products stay in range ((301/4)^16 < bf16 max, 4^-16 >> min).
fp16 s quantization adds ~3e-5 relative error to the loss.
"""
import sys

if "/opt/trn_rl_repo" not in sys.path:
    sys.path.insert(0, "/opt/trn_rl_repo")

import math

import numpy as np

import concourse.bacc as bacc
import concourse.bass as bass
import concourse.mybir as mybir
import concourse.tile as tile
from concourse.alu_op_type import AluOpType
from concourse.hw_specs import get_activation_tables

N = 16777216
N_CORES = 8
P = 128
ELEMS = N // N_CORES          # 2097152 per core
SCOLS = ELEMS // P            # 16384 fp16 s columns per partition row
BCOLS = ELEMS // P // 16      # 1024 u16 columns of bit-packed labels
TILES = [2048, 4096, 5120, 5120]
assert sum(TILES) == SCOLS and all(f % 16 == 0 for f in TILES)
GK = 16                       # group size of the product tree
ZCOLS = SCOLS // GK           # 1024 ln inputs per partition
SCALE = 0.25                  # per-element scale inside the products
LN_OFF = N * math.log(1.0 / SCALE)  # host-side offset: sum softplus = sum ln z + LN_OFF

_NC_CACHE = None


def _light_drain_and_barrier(self, tick_clock, wait_clock):
    """TileContext exit with the semaphore-clear cascade and second barrier
    dropped: the Bass preamble re-clears semaphores on each launch, so the
    exit-side clear is redundant for this kernel."""
    from concourse.tile import ScopedClock

    drain_inst = self.nc.sync.drain()
    wait_clock.add_sem_waits(drain_inst.ins, ScopedClock({None: tick_clock.global_clock}))
    self.nc.all_engine_barrier()
    assert self.sems is not None
    popped = self.nc._tile_sem_poison_stack.pop()
    assert popped is self._sem_poison
    self.nc._tile_sem_poison_stack  # keep attribute referenced


def build_nc(tiles=None):
    tiles = TILES if tiles is None else tiles
    nc = bacc.Bacc(
        "TRN2",
        target_bir_lowering=False,
        debug=False,
        enable_asserts=False,
        num_devices=N_CORES,
    )
    # One input tensor: cols [0:BCOLS) = bit-packed labels (u16),
    # cols [BCOLS : BCOLS+SCOLS) = s as fp16 (bitcast from u16).
    data_dram = nc.dram_tensor(
        "data", [P, BCOLS + SCOLS], mybir.dt.uint16, kind="ExternalInput"
    ).ap()
    out_dram = nc.dram_tensor(
        "partials", [1, 3], mybir.dt.float32, kind="ExternalOutput"
    ).ap()

    orig_drain = tile.TileContext._drain_and_barrier
    tile.TileContext._drain_and_barrier = _light_drain_and_barrier
    try:
        _build_body(nc, tiles, data_dram, out_dram)
    finally:
        tile.TileContext._drain_and_barrier = orig_drain
    nc.compile()
    return nc


def _build_body(nc, tiles, data_dram, out_dram):
    T = len(tiles)
    fmax = max(tiles)
    with tile.TileContext(nc) as tc:
        # Preload the table set that contains BOTH exp and ln so the
        # insertion pass emits no per-tile reloads.
        act_tables = list(get_activation_tables(nc.m.arch).keys())
        nle_id = act_tables.index("natural_log_exp_and_others")
        nc.scalar.add_instruction(mybir.InstLoadActFuncSet(
            name=nc.get_next_instruction_name(), ins=[], outs=[],
            act_func_set_id=nle_id,
        ))
        with tc.tile_pool(name="io", bufs=4) as io_pool, \
             tc.tile_pool(name="ejunk", bufs=2) as e_pool, \
             tc.tile_pool(name="tjunk", bufs=2) as t_pool, \
             tc.tile_pool(name="pjunk", bufs=2) as pc_pool, \
             tc.tile_pool(name="psum", bufs=1, space="PSUM") as psum_pool, \
             tc.tile_pool(name="acc", bufs=1) as acc_pool:
            zstage = acc_pool.tile([P, ZCOLS], mybir.dt.bfloat16)
            sums = acc_pool.tile([P, 3], mybir.dt.float32)
            sb_out = acc_pool.tile([1, 3], mybir.dt.float32)
            ones_f = acc_pool.tile([P, 1], mybir.dt.float32)
            lnjunk = acc_pool.tile([P, ZCOLS], mybir.dt.bfloat16)
            pc_dummy = acc_pool.tile([P, 1], mybir.dt.uint16)
            bits_t = acc_pool.tile([P, BCOLS], mybir.dt.uint16)
            nc.gpsimd.memset(sums[:], 0.0)
            nc.gpsimd.memset(ones_f[:], 1.0)
            psum_t = psum_pool.tile([1, 3], mybir.dt.float32)

            # --- input DMAs: bits first, then the s tiles (ramped sizes) ---
            nc.sync.dma_start(bits_t[:], data_dram[:, 0:BCOLS])
            data_ts = []
            c0 = BCOLS
            for F in tiles:
                dt_ = io_pool.tile([P, fmax], mybir.dt.uint16, name="data_t")
                nc.sync.dma_start(dt_[:, 0:F], data_dram[:, c0:c0 + F])
                data_ts.append(dt_)
                c0 += F

            # --- SWAR popcount of the label bits (DVE, interleaved with the
            # product-tree work; only depends on the first small DMA) ---
            # v1 = x - ((x>>1)&0x5555); v2 = (v1&0x3333)+((v1>>2)&0x3333)
            # v3 = (v2&0x0F0F)+((v2>>4)&0x0F0F)  -> per-byte counts
            # pos = sum(v3&0xFF) + sum(v3>>8)    -> two accum columns
            def swar_ops():
                x = bits_t[:]
                t1 = pc_pool.tile([P, BCOLS], mybir.dt.uint16, name="pc_t")
                yield lambda: nc.vector.tensor_scalar(
                    out=t1[:], in0=x, scalar1=1, scalar2=0x5555,
                    op0=AluOpType.logical_shift_right, op1=AluOpType.bitwise_and)
                v1 = pc_pool.tile([P, BCOLS], mybir.dt.uint16, name="pc_v")
                yield lambda: nc.vector.tensor_tensor(
                    out=v1[:], in0=x, in1=t1[:], op=AluOpType.subtract)
                t2 = pc_pool.tile([P, BCOLS], mybir.dt.uint16, name="pc_t")
                yield lambda: nc.vector.tensor_scalar(
                    out=t2[:], in0=v1[:], scalar1=2, scalar2=0x3333,
                    op0=AluOpType.logical_shift_right, op1=AluOpType.bitwise_and)
                v2a = pc_pool.tile([P, BCOLS], mybir.dt.uint16, name="pc_v")
                yield lambda: nc.vector.tensor_scalar(
                    out=v2a[:], in0=v1[:], scalar1=0x3333, scalar2=None,
                    op0=AluOpType.bitwise_and)
                v2 = pc_pool.tile([P, BCOLS], mybir.dt.uint16, name="pc_t")
                yield lambda: nc.vector.tensor_tensor(
                    out=v2[:], in0=v2a[:], in1=t2[:], op=AluOpType.add)
                t3 = pc_pool.tile([P, BCOLS], mybir.dt.uint16, name="pc_v")
                yield lambda: nc.vector.tensor_scalar(
                    out=t3[:], in0=v2[:], scalar1=4, scalar2=0x0F0F,
                    op0=AluOpType.logical_shift_right, op1=AluOpType.bitwise_and)
                v3a = pc_pool.tile([P, BCOLS], mybir.dt.uint16, name="pc_t")
                yield lambda: nc.vector.tensor_scalar(
                    out=v3a[:], in0=v2[:], scalar1=0x0F0F, scalar2=None,
                    op0=AluOpType.bitwise_and)
                v3 = pc_pool.tile([P, BCOLS], mybir.dt.uint16, name="pc_v")
                yield lambda: nc.vector.tensor_tensor(
                    out=v3[:], in0=v3a[:], in1=t3[:], op=AluOpType.add)
                # accum ops only allow arith op0, so sum v3 (= lo + 256*hi)
                # and hi separately; host un-mixes: pos = sum(v3) - 255*sum(hi)
                t4 = pc_pool.tile([P, BCOLS], mybir.dt.uint16, name="pc_t")
                yield lambda: nc.vector.tensor_scalar(
                    out=t4[:], in0=v3[:], scalar1=8, scalar2=None,
                    op0=AluOpType.logical_shift_right)
                yield lambda: nc.vector.tensor_scalar(
                    out=pc_dummy.broadcast_to((P, BCOLS)), in0=v3[:],
                    scalar1=1.0, scalar2=None, op0=AluOpType.mult,
                    op1=AluOpType.add, accum_out=sums[:, 1:2])
                yield lambda: nc.vector.tensor_scalar(
                    out=pc_dummy.broadcast_to((P, BCOLS)), in0=t4[:],
                    scalar1=1.0, scalar2=None, op0=AluOpType.mult,
                    op1=AluOpType.add, accum_out=sums[:, 2:3])

            swar = swar_ops()

            def emit_swar(k):
                for _ in range(k):
                    op = next(swar, None)
                    if op is not None:
                        op()

            # front-load part of the popcount into DVE's ramp-up idle time
            emit_swar(4)

            # --- per-tile: exp + product tree down to groups of 16 ---
            z0 = 0
            for i, F in enumerate(tiles):
                s_t = data_ts[i][:, 0:F].bitcast(mybir.dt.float16)
                e_t = e_pool.tile([P, fmax], mybir.dt.bfloat16, name="e_t")
                nc.scalar.activation(e_t[:, 0:F], s_t,
                                     mybir.ActivationFunctionType.Exp)
                h = F // 2
                # uh = e_hi/16 + 1/16 ; v = (e_lo + 1) * uh   (= pair prod /16)
                uh = t_pool.tile([P, fmax // 2], mybir.dt.bfloat16, name="uh_t")
                nc.vector.tensor_scalar(
                    out=uh[:, 0:h], in0=e_t[:, h:F],
                    scalar1=SCALE * SCALE, scalar2=SCALE * SCALE,
                    op0=AluOpType.mult, op1=AluOpType.add)
                v = t_pool.tile([P, fmax // 2], mybir.dt.bfloat16, name="v_t")
                nc.vector.scalar_tensor_tensor(
                    out=v[:, 0:h], in0=e_t[:, 0:h], scalar=1.0, in1=uh[:, 0:h],
                    op0=AluOpType.add, op1=AluOpType.mult)
                w = t_pool.tile([P, fmax // 4], mybir.dt.bfloat16, name="w_t")
                nc.vector.tensor_tensor(
                    out=w[:, 0:F // 4], in0=v[:, 0:F // 4], in1=v[:, F // 4:h],
                    op=AluOpType.mult)
                q = t_pool.tile([P, fmax // 8], mybir.dt.bfloat16, name="q_t")
                nc.vector.tensor_tensor(
                    out=q[:, 0:F // 8], in0=w[:, 0:F // 8], in1=w[:, F // 8:F // 4],
                    op=AluOpType.mult)
                nz = F // GK
                nc.vector.tensor_tensor(
                    out=zstage[:, z0:z0 + nz], in0=q[:, 0:nz], in1=q[:, nz:F // 8],
                    op=AluOpType.mult)
                z0 += nz
                emit_swar(2)
            emit_swar(10)  # drain any remaining popcount ops

            # --- tail: one ln over all group products, partition reduce ---
            nc.scalar.activation(
                lnjunk[:], zstage[:], mybir.ActivationFunctionType.Ln,
                accum_out=sums[:, 0:1])
            nc.tensor.matmul(psum_t[:], ones_f[:], sums[:],
                             start=True, stop=True)
            nc.vector.tensor_copy(sb_out[:], psum_t[:])
            nc.sync.dma_start(out_dram[:], sb_out[:])


def get_nc():
    global _NC_CACHE
    if _NC_CACHE is None:
        _NC_CACHE = build_nc()
    return _NC_CACHE


def shard_inputs(predicted_values, labels):
    """Pack per-core inputs: u16 [P, BCOLS+SCOLS] = bitpacked labels | fp16 s
    with the label XOR'd into the sign bit of s."""
    pv = np.ascontiguousarray(predicted_values, dtype=np.float32).reshape(N_CORES, -1)
    lb = np.ascontiguousarray(labels, dtype=np.int32).reshape(N_CORES, -1)
    s16 = pv.astype(np.float16).view(np.uint16)
    s16 = s16 ^ (lb.astype(np.uint16) << 15)
    data = np.empty((N_CORES, P, BCOLS + SCOLS), dtype=np.uint16)
    data[:, :, BCOLS:] = s16.reshape(N_CORES, P, SCOLS)
    bits = np.packbits(
        lb.astype(np.uint8).reshape(N_CORES, P, SCOLS, 1), axis=2, bitorder="little"
    )  # [cores, P, SCOLS//8, 1] u8
    data[:, :, :BCOLS] = (
        bits.reshape(N_CORES, P, SCOLS // 8).view(np.uint16)
    )
    return [{"data": data[c]} for c in range(N_CORES)]


def combine(results):
    """results: list of 8 dicts with 'partials' [1,3] f32 -> loss [1] f32.

    col 0: sum(ln z); col 1: sum(v3)=lo+256*hi; col 2: sum(hi)."""
    S = 0.0
    pos = 0.0
    for r in results:
        part = r["partials"].astype(np.float64)
        S += part[0, 0]
        pos += part[0, 1] - 255.0 * part[0, 2]
    S += LN_OFF
    neg = float(N) - pos
    loss = S / ((1.0 + neg) * pos)
    return np.array([loss], dtype=np.float32)


_RUNNER = None


def _get_runner():
    """Build the SPMD executable ONCE and reuse it (run_bass_kernel_spmd
    re-jits every call)."""
    global _RUNNER
    if _RUNNER is not None:
        return _RUNNER
    import jax
    from jax.sharding import Mesh, PartitionSpec
    from jax.experimental.shard_map import shard_map

    from concourse import bass2jax, mybir as mb

    nc = get_nc()
    bass2jax.install_neuronx_cc_hook()
    assert nc.dbg_addr is None
    partition_name = nc.partition_id_tensor.name if nc.partition_id_tensor else None

    in_names, out_names, out_avals, zero_outs = [], [], [], []
    for alloc in nc.m.functions[0].allocations:
        if not isinstance(alloc, mb.MemoryLocationSet):
            continue
        name = alloc.memorylocations[0].name
        if alloc.kind == "ExternalInput":
            if name != partition_name:
                in_names.append(name)
        elif alloc.kind == "ExternalOutput":
            shape = tuple(alloc.tensor_shape)
            dtype = mb.dt.np(alloc.dtype)
            out_names.append(name)
            out_avals.append(jax.core.ShapedArray(shape, dtype))
            zero_outs.append(np.zeros(shape, dtype))
    n_params = len(in_names)
    donate = tuple(range(n_params, n_params + len(out_avals)))
    all_in_names = list(in_names) + list(out_names)
    if partition_name is not None:
        all_in_names.append(partition_name)

    def _body(*args):
        operands = list(args)
        if partition_name is not None:
            operands.append(bass2jax.partition_id_tensor())
        outs = bass2jax._bass_exec_p.bind(
            *operands,
            out_avals=tuple(out_avals),
            in_names=tuple(all_in_names),
            out_names=tuple(out_names),
            lowering_input_output_aliases=(),
            sim_require_finite=True,
            sim_require_nnan=True,
            nc=nc,
        )
        return tuple(outs)

    devices = jax.devices()[:N_CORES]
    mesh = Mesh(np.asarray(devices), ("core",))
    nio = n_params + len(out_avals)
    sharded = jax.jit(
        shard_map(
            _body,
            mesh=mesh,
            in_specs=(PartitionSpec("core"),) * nio,
            out_specs=(PartitionSpec("core"),) * len(out_names),
            check_rep=False,
        ),
        donate_argnums=donate,
        keep_unused=True,
    )

    def run(in_maps):
        concat_in = [
            np.concatenate([np.asarray(m[name]) for m in in_maps], axis=0)
            for name in in_names
        ]
        concat_zeros = [
            np.zeros((N_CORES * z.shape[0], *z.shape[1:]), z.dtype)
            for z in zero_outs
        ]
        out_arrs = sharded(*concat_in, *concat_zeros)
        return [
            {
                name: np.asarray(out_arrs[k]).reshape(N_CORES, *out_avals[k].shape)[c]
                for k, name in enumerate(out_names)
            }
            for c in range(N_CORES)
        ]

    _RUNNER = run
    return _RUNNER


def kernel(predicted_values, labels):
    assert predicted_values.shape == (N,) and labels.shape == (N,)
    in_maps = shard_inputs(predicted_values, labels)
    results = _get_runner()(in_maps)
    return combine(results)


if __name__ == "__main__":
    rng = np.random.default_rng(0)
    pv = rng.standard_normal(N).astype(np.float32)
    lb = rng.integers(0, 2, size=N).astype(np.int32)
    out = kernel(pv, lb)
    print("loss:", out)
